# revision 21
# baseline (speedup 1.0000x reference)
"""Trainium2 Bass kernel for nn_Attention_1322849927460.

Dense transformer block: LN -> qkv -> attention (+ spatial-bias MLP on
attention-weighted coordinate deltas) -> out proj -> gelu -> residual.

Sharding: 8 cores = (2 batches) x (4 sequence quarters). Each core holds
all 8 heads for its 512 query rows and the full 2048-token K/V of its
batch, so no collectives are needed. A host-side roll of the token axis
puts each core's query rows first, letting all cores run an identical
SPMD program (attention is invariant to key-order permutation).

Algebraic structure:
  * delta_full[b,h,i,:] = (attn @ xyz)[b,h,i,:] - xyz[b,i,:] since softmax
    rows sum to one -> the (m,m,3) delta tensor is never formed.
  * softmax denominators come free from an augmented V' = [V | xyz | 1]
    contraction; one reciprocal + partition-broadcast normalizes the
    [68, i] accumulator at the end.
  * ln_g and the 1/sqrt(dh) q-scale fold into the qkv weights on host.

Engine split (the point of this version vs the naive schedule):
  * LayerNorm is pipelined per 4-tile group so transposes/qkv chase the
    stats instead of waiting for all 16 tiles.
  * exp of the attention logits alternates between the scalar engine
    (exact ACTIVATE) and the vector engine (Schraudolph fast exp:
    i16 = round(x*128/ln2 + (16256-5.5)), bitcast to bf16), doubling
    softmax throughput. QK logits land in bf16 PSUM so the DVE runs in
    its 2x packed mode.
  * v/qt evacuations run on the scalar engine during the (otherwise
    scalar-idle) LN phase; normalization multiplies run on gpsimd.
  * the spatial-MLP first layer (K=3) packs 4 heads into the PE array
    via tile_position row tiling; gelu is one batched 2048-elem
    ACTIVATE per head.
  * all DMAs issue from the sync queue, keeping the scalar engine free
    for exp/gelu.
"""

import os
import sys

for _p in ("/opt/trn_rl_repo",):
    if _p not in sys.path and os.path.isdir(_p):
        sys.path.insert(0, _p)

import ml_dtypes
import numpy as np

import concourse.bass as bass
import concourse.bacc as bacc
import concourse.tile as tile
from concourse import mybir
from concourse.bass_utils import run_bass_kernel_spmd
from concourse.masks import make_identity

F32 = mybir.dt.float32
BF16 = mybir.dt.bfloat16
I16 = mybir.dt.int16
AF = mybir.ActivationFunctionType
OP = mybir.AluOpType
BF = ml_dtypes.bfloat16

DIM = 256
H = 8
DH = 64
INNER = H * DH  # 512
M = 2048  # tokens per batch
TQ = 512  # query tokens per core
NT = M // 128  # 16 token tiles
N_CORES = 8
LN_EPS = 1e-5

# Schraudolph fast exp in bf16 bit domain:
#   bf16(x) bits = round(x * 2^7/ln2 + (127*2^7 - C)) viewed as int16.
EXP_A = 128.0 / float(np.log(2.0))
EXP_B = 16256.0 - 5.5


def build_program(has_bqkv: bool, has_spb1: bool, has_spb2: bool):
    nc = bacc.Bacc()

    x_d = nc.dram_tensor("x", [M, DIM], BF16, kind="ExternalInput")
    xyzv_d = nc.dram_tensor("xyzv", [128, NT, 4], BF16, kind="ExternalInput")
    xyzt_d = nc.dram_tensor("xyzt", [3, TQ], BF16, kind="ExternalInput")
    featt_d = nc.dram_tensor("featt", [DIM, TQ], F32, kind="ExternalInput")
    wqkv_d = nc.dram_tensor("wqkv", [DIM, 3 * INNER], BF16, kind="ExternalInput")
    spw1_d = nc.dram_tensor("spw1", [3, 2 * DIM], BF16, kind="ExternalInput")
    spw2_d = nc.dram_tensor("spw2", [2 * DIM, DH], BF16, kind="ExternalInput")
    wout_d = nc.dram_tensor("wout", [64, H, DIM], BF16, kind="ExternalInput")
    cf32_d = nc.dram_tensor("cf32", [128, 16], F32, kind="ExternalInput")
    cbf_d = nc.dram_tensor("cbf", [1, TQ + INNER + DH], BF16, kind="ExternalInput")
    out_d = nc.dram_tensor("out", [DIM, TQ], F32, kind="ExternalOutput")

    with tile.TileContext(nc) as tc:
        with (
            tc.tile_pool(name="const", bufs=1) as constp,
            tc.tile_pool(name="big", bufs=1) as bigp,
            tc.tile_pool(name="work", bufs=2) as workp,
        ):
            # ---- DMAs: all on the sync HWDGE queue, critical-path first.
            wqkv_sb = constp.tile([128, 2, 3 * INNER], BF16)
            nc.sync.dma_start(
                out=wqkv_sb, in_=wqkv_d[:].rearrange("(cc p) o -> p cc o", p=128)
            )
            x_sb = bigp.tile([128, NT, DIM], BF16)
            xv = x_d[:].rearrange("(n p) c -> p n c", p=128)
            for g in range(4):
                nc.sync.dma_start(
                    out=x_sb[:, 4 * g : 4 * g + 4, :],
                    in_=xv[:, 4 * g : 4 * g + 4, :],
                )
            xyzv_sb = constp.tile([128, NT, 4], BF16)
            nc.sync.dma_start(out=xyzv_sb, in_=xyzv_d[:])
            xyzt_sb = constp.tile([67, TQ], BF16)
            nc.sync.dma_start(out=xyzt_sb[64:67, :], in_=xyzt_d[:])
            cbf_sb = constp.tile([1, TQ + INNER + DH], BF16)
            nc.sync.dma_start(out=cbf_sb, in_=cbf_d[:])
            cf32_sb = constp.tile([128, 16], F32)
            nc.sync.dma_start(out=cf32_sb, in_=cf32_d[:])
            spw1_sb = constp.tile([67, 2 * DIM], BF16)
            nc.sync.dma_start(out=spw1_sb[64:67, :], in_=spw1_d[:])
            spw2_sb = constp.tile([128, 4, DH], BF16)
            nc.sync.dma_start(
                out=spw2_sb, in_=spw2_d[:].rearrange("(kc p) d -> p kc d", p=128)
            )
            wout_sb = constp.tile([64, H, DIM], BF16)
            nc.sync.dma_start(out=wout_sb, in_=wout_d[:])
            featt_sb = constp.tile([128, 2, TQ], F32)
            nc.sync.dma_start(
                out=featt_sb, in_=featt_d[:].rearrange("(ec p) t -> p ec t", p=128)
            )

            ones_tq = cbf_sb[0:1, 0:TQ]
            bv_sb = cbf_sb[0:1, TQ : TQ + INNER]
            spb2_sb = cbf_sb[0:1, TQ + INNER : TQ + INNER + DH]
            bqk_sb = cf32_sb[:, 0:8]
            spb1_sb = cf32_sb[:, 8:12]
            outb_sb = cf32_sb[:, 12:14]

            ident = constp.tile([128, 128], BF16)
            make_identity(nc, ident)
            eps_t = constp.tile([128, 1], F32)
            nc.vector.memset(eps_t, LN_EPS)

            # xyz|ones columns of Vaug: one compact DMA + strided gpsimd
            # copies per head (avoids a 16k-packet strided DMA).
            vaug_sb = bigp.tile([128, NT, H, DH + 4], BF16)
            for h in range(H):
                nc.gpsimd.tensor_copy(vaug_sb[:, :, h, DH : DH + 4], xyzv_sb)

            # PE priming: absorb one DMA-queue semaphore per DMA-loaded
            # tile the PE consumes, so real matmuls stay under the
            # per-instruction sync-wait limit. Plus warm spam to open the
            # HAM clock gate during the DMA lead-in.
            # Pool nesting (LIFO): kb outlives ptr/pqv (attention reuses it).
            kb_cm = tc.tile_pool(name="kb", bufs=1, space="PSUM")
            kb = kb_cm.__enter__()
            ptr_cm = tc.tile_pool(name="ptr", bufs=2, space="PSUM")
            ptr = ptr_cm.__enter__()
            warm_ps = ptr.tile([128, 128], BF16, tag="warm", bufs=1)

            def warm(n):
                for _ in range(n):
                    nc.tensor.transpose(warm_ps, ident, ident)

            warm(24)
            prime_ps = ptr.tile([4, 4], F32, tag="prime", bufs=1)

            def prime(lhsT, rhs):
                nc.tensor.matmul(
                    prime_ps[0 : lhsT.shape[-1], 0 : rhs.shape[-1]],
                    lhsT,
                    rhs,
                    start=True,
                    stop=True,
                )

            prime(wqkv_sb[:, 0, 0:4], wqkv_sb[:, 0, 0:4])
            prime(spw1_sb[64:67, 0:4], spw1_sb[64:67, 0:4])
            prime(spw2_sb[:, 0, 0:4], spw2_sb[:, 0, 0:4])
            prime(wout_sb[:, 0, 0:4], wout_sb[:, 0, 0:4])
            if has_bqkv:
                prime(ones_tq[:, 0:4], bv_sb[:, 0:4])
            if has_spb2:
                prime(spb2_sb[:, 0:4], ones_tq[:, 0:4])
            warm(12)

            # ---- Phase A: LN -> transpose -> q/kt0/v, pipelined per
            # 4-tile group.
            xn_sb = bigp.tile([128, NT, DIM], BF16)
            xnt_sb = bigp.tile([128, 2, M], BF16)
            qt_sb = bigp.tile([128, 4, TQ], BF16)
            kt_sb = bigp.tile([128, 4, M], BF16)
            mv_all = constp.tile([128, NT, 2], F32)
            rstd = constp.tile([128, NT], F32)

            pqv_cm = tc.tile_pool(name="pqv", bufs=2, space="PSUM")
            pqv = pqv_cm.__enter__()

            for g in range(4):
                for q in range(4):
                    n = 4 * g + q
                    stats = workp.tile([128, 6], F32, tag="bnstats")
                    nc.vector.bn_stats(out=stats, in_=x_sb[:, n, :])
                    nc.vector.bn_aggr(out=mv_all[:, n, :], in_=stats)
                nc.scalar.activation(
                    out=rstd[:, 4 * g : 4 * g + 4],
                    in_=mv_all[:, 4 * g : 4 * g + 4, 1],
                    func=AF.Sqrt,
                    bias=eps_t,
                    scale=1.0,
                )
                nc.vector.reciprocal(
                    out=rstd[:, 4 * g : 4 * g + 4],
                    in_=rstd[:, 4 * g : 4 * g + 4],
                )
                for q in range(4):
                    n = 4 * g + q
                    nc.vector.tensor_scalar(
                        out=xn_sb[:, n, :],
                        in0=x_sb[:, n, :],
                        scalar1=mv_all[:, n, 0:1],
                        scalar2=rstd[:, n : n + 1],
                        op0=OP.subtract,
                        op1=OP.mult,
                    )
                # transpose this group into xnT
                for cc in range(2):
                    ps = ptr.tile([128, 512], BF16, tag="tr")
                    for q in range(4):
                        n = 4 * g + q
                        nc.tensor.transpose(
                            ps[:, q * 128 : (q + 1) * 128],
                            xn_sb[:, n, cc * 128 : (cc + 1) * 128],
                            ident,
                        )
                    nc.vector.tensor_copy(
                        xnt_sb[:, cc, g * 512 : (g + 1) * 512], ps
                    )
                if g == 0:
                    # q projection for this core's 512 queries; borrows
                    # the kb pool's banks (emit_kt only runs after g3).
                    for grp in range(2):
                        ps_q = kb.tile([128, 2, TQ], F32, tag="k", bufs=1)
                        for oo in range(2):
                            oc = grp * 2 + oo
                            for cc in range(2):
                                nc.tensor.matmul(
                                    ps_q[:, oo, :],
                                    wqkv_sb[:, cc, oc * 128 : (oc + 1) * 128],
                                    xnt_sb[:, cc, 0:TQ],
                                    start=(cc == 0),
                                    stop=(cc == 1),
                                )
                        for oo in range(2):
                            oc = grp * 2 + oo
                            if has_bqkv:
                                nc.vector.tensor_scalar(
                                    out=qt_sb[:, oc, :],
                                    in0=ps_q[:, oo, :],
                                    scalar1=bqk_sb[:, oc : oc + 1],
                                    scalar2=None,
                                    op0=OP.add,
                                )
                            else:
                                nc.vector.tensor_copy(
                                    qt_sb[:, oc, :], ps_q[:, oo, :]
                                )
                # v for this group: evacuate on the scalar engine (idle
                # during LN; the vector engine is the phase-A bottleneck).
                for q in range(4):
                    n = 4 * g + q
                    ps_v = pqv.tile([128, INNER], F32, tag="v", bufs=2)
                    for cc in range(2):
                        nc.tensor.matmul(
                            ps_v,
                            xnt_sb[:, cc, n * 128 : (n + 1) * 128],
                            wqkv_sb[:, cc, 2 * INNER : 3 * INNER],
                            start=(cc == 0),
                            stop=(cc == 1 and not has_bqkv),
                        )
                    if has_bqkv:
                        nc.tensor.matmul(
                            ps_v, ones_tq[:, 0:128], bv_sb, start=False, stop=True
                        )
                    nc.scalar.activation(
                        out=vaug_sb[:, n, :, 0:DH],
                        in_=ps_v[:].rearrange("p (h d) -> p h d", h=H),
                        func=AF.Copy,
                    )

            def emit_kt(oc):
                for half in range(2):
                    ps_k = kb.tile([128, 2, TQ], F32, tag="k", bufs=1)
                    for tt in range(2):
                        tb = half * 2 + tt
                        for cc in range(2):
                            nc.tensor.matmul(
                                ps_k[:, tt, :],
                                wqkv_sb[
                                    :, cc, INNER + oc * 128 : INNER + (oc + 1) * 128
                                ],
                                xnt_sb[:, cc, tb * 512 : (tb + 1) * 512],
                                start=(cc == 0),
                                stop=(cc == 1),
                            )
                    if has_bqkv:
                        nc.vector.tensor_scalar(
                            out=kt_sb[:, oc, half * 1024 : (half + 1) * 1024],
                            in0=ps_k,
                            scalar1=bqk_sb[:, 4 + oc : 5 + oc],
                            scalar2=None,
                            op0=OP.add,
                        )
                    elif half == 0:
                        nc.vector.tensor_copy(
                            kt_sb[:, oc, half * 1024 : (half + 1) * 1024], ps_k
                        )
                    else:
                        nc.scalar.activation(
                            out=kt_sb[:, oc, half * 1024 : (half + 1) * 1024],
                            in_=ps_k,
                            func=AF.Copy,
                        )

            emit_kt(0)
            pqv_cm.__exit__(None, None, None)
            ptr_cm.__exit__(None, None, None)

            # ---- attention: 4 passes x 2 heads ----
            # Unnormalized accumulators are evicted per pass; exp
            # alternates scalar/vector by j parity; kT chunks 1-3 are
            # emitted inside earlier passes to fill PE gaps.
            araw_sb = bigp.tile([68, 4, 2, TQ], F32)
            an_sb = bigp.tile([64, 4, 2, TQ], BF16)
            dnp_sb = bigp.tile([67, 4, 2, TQ], BF16)
            rsp_cm = tc.tile_pool(name="rsp", bufs=2)
            rsp = rsp_cm.__enter__()
            with (
                tc.tile_pool(name="pattn", bufs=2, space="PSUM") as pattn,
                tc.tile_pool(name="expp", bufs=2) as expp,
            ):
                for p in range(4):
                    accum = pattn.tile([68, 2, TQ], F32, tag="accum", bufs=1)
                    for j in range(NT):
                        sT = pattn.tile([128, 2, TQ], F32, tag="sT", bufs=2)
                        for hh in range(2):
                            nc.tensor.matmul(
                                sT[:, hh, :],
                                kt_sb[
                                    hh * 64 : hh * 64 + 64,
                                    p,
                                    j * 128 : (j + 1) * 128,
                                ],
                                qt_sb[hh * 64 : hh * 64 + 64, p, :],
                                start=True,
                                stop=True,
                            )
                        # split exp by head so the two halves run
                        # concurrently on scalar + vector: halves the exp
                        # latency on the AV critical path.
                        e = expp.tile([128, TQ], BF16, tag="es", bufs=2)
                        nc.scalar.activation(
                            out=e, in_=sT[:, 0, :], func=AF.Exp
                        )
                        ei = expp.tile([128, TQ], I16, tag="ev", bufs=2)
                        nc.vector.tensor_scalar(
                            out=ei,
                            in0=sT[:, 1, :],
                            scalar1=EXP_A,
                            scalar2=EXP_B,
                            op0=OP.mult,
                            op1=OP.add,
                        )
                        eaps = [e[:], ei[:].bitcast(BF16)]
                        for hh in range(2):
                            h = 2 * p + hh
                            nc.tensor.matmul(
                                accum[:, hh, :],
                                vaug_sb[:, j, h, :],
                                eaps[hh],
                                start=(j == 0),
                                stop=(j == NT - 1),
                            )
                        if j == 5 and p < 3:
                            emit_kt(p + 1)
                    nc.vector.tensor_copy(araw_sb[:, p, :, :], accum)
                    # normalization runs under the next pass: reciprocal
                    # of the ones-row via a small DMA gather, broadcast,
                    # multiply on gpsimd, xyz-subtract, and a tiny DMA
                    # restack of the 3 delta rows to 32*i offsets for the
                    # row-packed MLP.
                    rs = rsp.tile([128, 8], F32, tag="rs")
                    nc.sync.dma_start(out=rs, in_=araw_sb[67:68, p, :, :])
                    rc = rsp.tile([128, 8], F32, tag="rc")
                    nc.vector.reciprocal(out=rc, in_=rs)
                    rrow = rsp.tile([1, 2, TQ], F32, tag="rrow")
                    nc.sync.dma_start(out=rrow, in_=rc)
                    for hh in range(2):
                        rbc = rsp.tile([68, TQ], F32, tag="rbc", bufs=3)
                        nc.gpsimd.partition_broadcast(
                            rbc, rrow[0:1, hh, :], channels=68
                        )
                        nc.vector.tensor_tensor(
                            out=an_sb[:, p, hh, :],
                            in0=araw_sb[0:64, p, hh, :],
                            in1=rbc[0:64, :],
                            op=OP.mult,
                        )
                        dn = dnp_sb[:, p, hh, :]
                        nc.vector.tensor_tensor(
                            out=dn[64:67, :],
                            in0=araw_sb[64:67, p, hh, :],
                            in1=rbc[64:67, :],
                            op=OP.mult,
                        )
                        nc.vector.tensor_tensor(
                            out=dn[64:67, :],
                            in0=dn[64:67, :],
                            in1=xyzt_sb[64:67, :],
                            op=OP.subtract,
                        )
            kb_cm.__exit__(None, None, None)

            # ---- spatial-bias MLP: kc-granular h1 with the h2
            # accumulation interleaved per kc, so the PE fills the gelu
            # shadow. 4 sbias accumulators stay live across the kc loop.
            outfin_sb = bigp.tile([64, H, TQ], BF16)
            with (
                tc.tile_pool(name="pmlp", bufs=1, space="PSUM") as pmlp,
                tc.tile_pool(name="hpool", bufs=2) as hpool,
            ):
                warm2 = pmlp.tile([128, 2, TQ], F32, tag="h1", bufs=2)
                wv = warm2[:].bitcast(BF16)
                for _ in range(20):
                    nc.tensor.transpose(wv[:, 0, 0:128], ident, ident)

                for G in range(2):
                    hsb_G = hpool.tile([128, 4, 4, TQ], BF16, tag="hsb")
                    sb_tiles = [
                        pmlp.tile(
                            [64, TQ], F32, name=f"sb{G}_{i}", tag="sbias", bufs=4
                        )
                        for i in range(4)
                    ]
                    for kc in range(4):
                        for pr in range(2):
                            h1 = pmlp.tile([128, 2, TQ], F32, tag="h1", bufs=2)
                            for ii in range(2):
                                i = 2 * pr + ii
                                h = 4 * G + i
                                ph, hhh = h // 2, h % 2
                                nc.tensor.matmul(
                                    h1[:, ii, :],
                                    spw1_sb[64:67, kc * 128 : (kc + 1) * 128],
                                    dnp_sb[64:67, ph, hhh, :],
                                    start=True,
                                    stop=True,
                                )
                            bias_kc = (
                                spb1_sb[:, kc : kc + 1] if has_spb1 else 0.0
                            )
                            nc.scalar.activation(
                                out=hsb_G[:, 2 * pr : 2 * pr + 2, kc, :],
                                in_=h1,
                                func=AF.Gelu,
                                bias=bias_kc,
                            )
                            for ii in range(2):
                                i = 2 * pr + ii
                                nc.tensor.matmul(
                                    sb_tiles[i],
                                    spw2_sb[:, kc, :],
                                    hsb_G[:, i, kc, :],
                                    start=(kc == 0),
                                    stop=(kc == 3 and not has_spb2),
                                )
                    for i in range(4):
                        h = 4 * G + i
                        if has_spb2:
                            nc.tensor.matmul(
                                sb_tiles[i], spb2_sb, ones_tq, start=False, stop=True
                            )
                        p, hh = h // 2, h % 2
                        nc.vector.tensor_tensor(
                            out=outfin_sb[:, h, :],
                            in0=an_sb[:, p, hh, :],
                            in1=sb_tiles[i],
                            op=OP.add,
                        )

            # ---- output projection + gelu + residual ----
            with tc.tile_pool(name="pproj", bufs=1, space="PSUM") as pproj:
                yT = pproj.tile([128, 2, TQ], F32, tag="y", bufs=1)
                for ec in range(2):
                    for h in range(H):
                        nc.tensor.matmul(
                            yT[:, ec, :],
                            wout_sb[:, h, ec * 128 : (ec + 1) * 128],
                            outfin_sb[:, h, :],
                            start=(h == 0),
                            stop=(h == H - 1),
                        )
                for ec in range(2):
                    ysb = workp.tile([128, TQ], F32, tag="ysb")
                    nc.scalar.activation(
                        out=ysb,
                        in_=yT[:, ec, :],
                        func=AF.Gelu,
                        bias=outb_sb[:, ec : ec + 1],
                    )
                    res = workp.tile([128, TQ], F32, tag="res")
                    nc.vector.tensor_tensor(
                        out=res, in0=ysb, in1=featt_sb[:, ec, :], op=OP.add
                    )
                    nc.sync.dma_start(
                        out=out_d[:].rearrange("(ec p) t -> p ec t", p=128)[:, ec, :],
                        in_=res,
                    )
            rsp_cm.__exit__(None, None, None)

    nc.compile()
    return nc


def prepare_maps(inputs):
    xyzs = np.asarray(inputs["xyzs"], np.float32)
    features = np.asarray(inputs["features"], np.float32)
    ln_g = np.asarray(inputs["ln_g"], np.float32)
    ln_b = np.asarray(inputs["ln_b"], np.float32)
    w_qkv = np.asarray(inputs["w_qkv"], np.float32)
    sp_w1 = np.asarray(inputs["sp_w1"], np.float32)
    sp_b1 = np.asarray(inputs["sp_b1"], np.float32)
    sp_w2 = np.asarray(inputs["sp_w2"], np.float32)
    sp_b2 = np.asarray(inputs["sp_b2"], np.float32)
    out_w = np.asarray(inputs["out_w"], np.float32)
    out_b = np.asarray(inputs["out_b"], np.float32)

    scale = DH ** -0.5
    wqkv_f = w_qkv * ln_g[:, None]
    wqkv_f[:, :INNER] = wqkv_f[:, :INNER] * scale
    bqkv = (ln_b @ w_qkv).astype(np.float32)
    bqkv[:INNER] *= scale

    has_bqkv = bool(np.any(bqkv != 0.0))
    has_spb1 = bool(np.any(sp_b1 != 0.0))
    has_spb2 = bool(np.any(sp_b2 != 0.0))

    cf32 = np.zeros((128, 16), np.float32)
    for oc in range(4):
        cf32[:, oc] = bqkv[oc * 128 : (oc + 1) * 128]
        cf32[:, 4 + oc] = bqkv[INNER + oc * 128 : INNER + (oc + 1) * 128]
    for kc in range(4):
        cf32[:, 8 + kc] = sp_b1[kc * 128 : (kc + 1) * 128]
    cf32[:, 12] = out_b[:128]
    cf32[:, 13] = out_b[128:]

    cbf = np.zeros((1, TQ + INNER + DH), np.float32)
    cbf[0, 0:TQ] = 1.0
    cbf[0, TQ : TQ + INNER] = bqkv[2 * INNER :]
    cbf[0, TQ + INNER :] = sp_b2


    # wout as [64, H, 256]: row (d, h) = out_w[h*64+d, :]
    wout64 = np.ascontiguousarray(out_w.reshape(H, 64, DIM).transpose(1, 0, 2))

    shared = {
        "wqkv": np.ascontiguousarray(wqkv_f).astype(BF),
        "cf32": cf32,
        "cbf": cbf.astype(BF),
        "spw1": np.ascontiguousarray(sp_w1).astype(BF),
        "spw2": np.ascontiguousarray(sp_w2).astype(BF),
        "wout": wout64.astype(BF),
    }

    in_maps = []
    for core in range(N_CORES):
        bi, quarter = core // 4, core % 4
        qs = quarter * TQ
        x_b = features[bi].reshape(M, DIM)
        xyz_b = xyzs[bi].reshape(M, 3)
        x_perm = np.roll(x_b, -qs, axis=0)
        xyz_perm = np.roll(xyz_b, -qs, axis=0)
        xyza = np.concatenate(
            [xyz_perm, np.ones((M, 1), np.float32)], axis=1
        ).astype(np.float32)
        m = dict(shared)
        m["x"] = np.ascontiguousarray(x_perm).astype(BF)
        m["xyzv"] = np.ascontiguousarray(
            xyza.reshape(NT, 128, 4).transpose(1, 0, 2)
        ).astype(BF)
        m["xyzt"] = np.ascontiguousarray(xyz_perm[:TQ].T).astype(BF)
        m["featt"] = np.ascontiguousarray(x_perm[:TQ].T)
        in_maps.append(m)
    return in_maps, (has_bqkv, has_spb1, has_spb2)


def assemble(results, l=16, n=128):
    out = np.zeros((2, M, DIM), np.float32)
    for core in range(N_CORES):
        bi, quarter = core // 4, core % 4
        qs = quarter * TQ
        out[bi, qs : qs + TQ, :] = results[core]["out"].T
    return out.reshape(2, l, n, DIM)


def kernel(**inputs):
    in_maps, flags = prepare_maps(inputs)
    nc = build_program(*flags)
    results = run_bass_kernel_spmd(nc, in_maps, list(range(N_CORES))).results
    return assemble(results)


if __name__ == "__main__":
    pass


# revision 22
# speedup vs baseline: 1.2996x; 1.2996x over previous
"""Trainium2 Bass kernel for nn_Attention_1322849927460.

Dense transformer block: LN -> qkv -> attention (+ spatial-bias MLP on
attention-weighted coordinate deltas) -> out proj -> gelu -> residual.

Sharding: 8 cores = (2 batches) x (4 sequence quarters). Each core holds
all 8 heads for its 512 query rows and the full 2048-token K/V of its
batch, so no collectives are needed. A host-side roll of the token axis
puts each core's query rows first, letting all cores run an identical
SPMD program (attention is invariant to key-order permutation).

Algebraic structure:
  * delta_full[b,h,i,:] = (attn @ xyz)[b,h,i,:] - xyz[b,i,:] since softmax
    rows sum to one -> the (m,m,3) delta tensor is never formed.
  * softmax denominators come free from an augmented V' = [V | xyz | 1]
    contraction; one reciprocal + partition-broadcast normalizes the
    [68, i] accumulator at the end.
  * ln_g and the 1/sqrt(dh) q-scale fold into the qkv weights on host.

Engine split (the point of this version vs the naive schedule):
  * LayerNorm is pipelined per 4-tile group so transposes/qkv chase the
    stats instead of waiting for all 16 tiles.
  * exp of the attention logits alternates between the scalar engine
    (exact ACTIVATE) and the vector engine (Schraudolph fast exp:
    i16 = round(x*128/ln2 + (16256-5.5)), bitcast to bf16), doubling
    softmax throughput. QK logits land in bf16 PSUM so the DVE runs in
    its 2x packed mode.
  * v/qt evacuations run on the scalar engine during the (otherwise
    scalar-idle) LN phase; normalization multiplies run on gpsimd.
  * the spatial-MLP first layer (K=3) packs 4 heads into the PE array
    via tile_position row tiling; gelu is one batched 2048-elem
    ACTIVATE per head.
  * all DMAs issue from the sync queue, keeping the scalar engine free
    for exp/gelu.
"""

import os
import sys

for _p in ("/opt/trn_rl_repo",):
    if _p not in sys.path and os.path.isdir(_p):
        sys.path.insert(0, _p)

import ml_dtypes
import numpy as np

import concourse.bass as bass
import concourse.bacc as bacc
import concourse.tile as tile
from concourse import mybir
from concourse.bass_utils import run_bass_kernel_spmd
from concourse.masks import make_identity

F32 = mybir.dt.float32
BF16 = mybir.dt.bfloat16
I16 = mybir.dt.int16
AF = mybir.ActivationFunctionType
OP = mybir.AluOpType
BF = ml_dtypes.bfloat16

DIM = 256
H = 8
DH = 64
INNER = H * DH  # 512
M = 2048  # tokens per batch
TQ = 512  # query tokens per core
NT = M // 128  # 16 token tiles
N_CORES = 8
LN_EPS = 1e-5

# Schraudolph fast exp in bf16 bit domain:
#   bf16(x) bits = round(x * 2^7/ln2 + (127*2^7 - C)) viewed as int16.
EXP_A = 128.0 / float(np.log(2.0))
EXP_B = 16256.0 - 5.5


def build_program(has_bqkv: bool, has_spb1: bool, has_spb2: bool):
    nc = bacc.Bacc()

    x_d = nc.dram_tensor("x", [M, DIM], BF16, kind="ExternalInput")
    xyzv_d = nc.dram_tensor("xyzv", [128, NT, 4], BF16, kind="ExternalInput")
    xyzt_d = nc.dram_tensor("xyzt", [3, TQ], BF16, kind="ExternalInput")
    featt_d = nc.dram_tensor("featt", [DIM, TQ], F32, kind="ExternalInput")
    wqkv_d = nc.dram_tensor("wqkv", [DIM, 3 * INNER], BF16, kind="ExternalInput")
    spw1_d = nc.dram_tensor("spw1", [3, 2 * DIM], BF16, kind="ExternalInput")
    spw2_d = nc.dram_tensor("spw2", [2 * DIM, DH], BF16, kind="ExternalInput")
    wout_d = nc.dram_tensor("wout", [64, H, DIM], BF16, kind="ExternalInput")
    cf32_d = nc.dram_tensor("cf32", [128, 16], F32, kind="ExternalInput")
    cbf_d = nc.dram_tensor("cbf", [1, TQ + INNER + DH], BF16, kind="ExternalInput")
    out_d = nc.dram_tensor("out", [DIM, TQ], F32, kind="ExternalOutput")

    with tile.TileContext(nc) as tc:
        with (
            tc.tile_pool(name="const", bufs=1) as constp,
            tc.tile_pool(name="big", bufs=1) as bigp,
            tc.tile_pool(name="work", bufs=2) as workp,
        ):
            # ---- DMAs: all on the sync HWDGE queue, critical-path first.
            wqkv_sb = constp.tile([128, 2, 3 * INNER], BF16)
            nc.sync.dma_start(
                out=wqkv_sb, in_=wqkv_d[:].rearrange("(cc p) o -> p cc o", p=128)
            )
            x_sb = bigp.tile([128, NT, DIM], BF16)
            xv = x_d[:].rearrange("(n p) c -> p n c", p=128)
            for g in range(4):
                nc.sync.dma_start(
                    out=x_sb[:, 4 * g : 4 * g + 4, :],
                    in_=xv[:, 4 * g : 4 * g + 4, :],
                )
            xyzv_sb = constp.tile([128, NT, 4], BF16)
            nc.sync.dma_start(out=xyzv_sb, in_=xyzv_d[:])
            xyzt_sb = constp.tile([67, TQ], BF16)
            nc.sync.dma_start(out=xyzt_sb[64:67, :], in_=xyzt_d[:])
            cbf_sb = constp.tile([1, TQ + INNER + DH], BF16)
            nc.sync.dma_start(out=cbf_sb, in_=cbf_d[:])
            cf32_sb = constp.tile([128, 16], F32)
            nc.sync.dma_start(out=cf32_sb, in_=cf32_d[:])
            spw1_sb = constp.tile([67, 2 * DIM], BF16)
            nc.sync.dma_start(out=spw1_sb[64:67, :], in_=spw1_d[:])
            spw2_sb = constp.tile([128, 4, DH], BF16)
            nc.sync.dma_start(
                out=spw2_sb, in_=spw2_d[:].rearrange("(kc p) d -> p kc d", p=128)
            )
            wout_sb = constp.tile([64, H, DIM], BF16)
            nc.sync.dma_start(out=wout_sb, in_=wout_d[:])
            featt_sb = constp.tile([128, 2, TQ], F32)
            nc.sync.dma_start(
                out=featt_sb, in_=featt_d[:].rearrange("(ec p) t -> p ec t", p=128)
            )

            ones_tq = cbf_sb[0:1, 0:TQ]
            bv_sb = cbf_sb[0:1, TQ : TQ + INNER]
            spb2_sb = cbf_sb[0:1, TQ + INNER : TQ + INNER + DH]
            bqk_sb = cf32_sb[:, 0:8]
            spb1_sb = cf32_sb[:, 8:12]
            outb_sb = cf32_sb[:, 12:14]

            ident = constp.tile([128, 128], BF16)
            make_identity(nc, ident)
            eps_t = constp.tile([128, 1], F32)
            nc.vector.memset(eps_t, LN_EPS)

            # xyz|ones columns of Vaug: one compact DMA + strided gpsimd
            # copies per head (avoids a 16k-packet strided DMA).
            vaug_sb = bigp.tile([128, NT, H, DH + 4], BF16)
            for h in range(H):
                nc.gpsimd.tensor_copy(vaug_sb[:, :, h, DH : DH + 4], xyzv_sb)

            # PE priming: absorb one DMA-queue semaphore per DMA-loaded
            # tile the PE consumes, so real matmuls stay under the
            # per-instruction sync-wait limit. Plus warm spam to open the
            # HAM clock gate during the DMA lead-in.
            # Pool nesting (LIFO): kb outlives ptr/pqv (attention reuses it).
            kb_cm = tc.tile_pool(name="kb", bufs=1, space="PSUM")
            kb = kb_cm.__enter__()
            ptr_cm = tc.tile_pool(name="ptr", bufs=2, space="PSUM")
            ptr = ptr_cm.__enter__()
            warm_ps = ptr.tile([128, 128], BF16, tag="warm", bufs=1)

            def warm(n):
                for _ in range(n):
                    nc.tensor.transpose(warm_ps, ident, ident)

            warm(24)
            prime_ps = ptr.tile([4, 4], F32, tag="prime", bufs=1)

            def prime(lhsT, rhs):
                nc.tensor.matmul(
                    prime_ps[0 : lhsT.shape[-1], 0 : rhs.shape[-1]],
                    lhsT,
                    rhs,
                    start=True,
                    stop=True,
                )

            prime(wqkv_sb[:, 0, 0:4], wqkv_sb[:, 0, 0:4])
            prime(spw1_sb[64:67, 0:4], spw1_sb[64:67, 0:4])
            prime(spw2_sb[:, 0, 0:4], spw2_sb[:, 0, 0:4])
            prime(wout_sb[:, 0, 0:4], wout_sb[:, 0, 0:4])
            if has_bqkv:
                prime(ones_tq[:, 0:4], bv_sb[:, 0:4])
            if has_spb2:
                prime(spb2_sb[:, 0:4], ones_tq[:, 0:4])
            warm(12)

            # ---- Phase A: LN -> transpose -> q/kt0/v, pipelined per
            # 4-tile group.
            xn_sb = bigp.tile([128, NT, DIM], BF16)
            xnt_sb = bigp.tile([128, 2, M], BF16)
            qt_sb = bigp.tile([128, 4, TQ], BF16)
            kt_sb = bigp.tile([128, 4, M], BF16)
            mv_all = constp.tile([128, NT, 2], F32)
            rstd = constp.tile([128, NT], F32)

            pqv_cm = tc.tile_pool(name="pqv", bufs=2, space="PSUM")
            pqv = pqv_cm.__enter__()

            for g in range(4):
                for q in range(4):
                    n = 4 * g + q
                    stats = workp.tile([128, 6], F32, tag="bnstats")
                    nc.vector.bn_stats(out=stats, in_=x_sb[:, n, :])
                    nc.vector.bn_aggr(out=mv_all[:, n, :], in_=stats)
                nc.scalar.activation(
                    out=rstd[:, 4 * g : 4 * g + 4],
                    in_=mv_all[:, 4 * g : 4 * g + 4, 1],
                    func=AF.Sqrt,
                    bias=eps_t,
                    scale=1.0,
                )
                nc.vector.reciprocal(
                    out=rstd[:, 4 * g : 4 * g + 4],
                    in_=rstd[:, 4 * g : 4 * g + 4],
                )
                for q in range(4):
                    n = 4 * g + q
                    nc.vector.tensor_scalar(
                        out=xn_sb[:, n, :],
                        in0=x_sb[:, n, :],
                        scalar1=mv_all[:, n, 0:1],
                        scalar2=rstd[:, n : n + 1],
                        op0=OP.subtract,
                        op1=OP.mult,
                    )
                # transpose this group into xnT
                for cc in range(2):
                    ps = ptr.tile([128, 512], BF16, tag="tr")
                    for q in range(4):
                        n = 4 * g + q
                        nc.tensor.transpose(
                            ps[:, q * 128 : (q + 1) * 128],
                            xn_sb[:, n, cc * 128 : (cc + 1) * 128],
                            ident,
                        )
                    nc.vector.tensor_copy(
                        xnt_sb[:, cc, g * 512 : (g + 1) * 512], ps
                    )
                if g == 0:
                    # q projection for this core's 512 queries; borrows
                    # the kb pool's banks (emit_kt only runs after g3).
                    for grp in range(2):
                        ps_q = kb.tile([128, 2, TQ], F32, tag="k", bufs=1)
                        for oo in range(2):
                            oc = grp * 2 + oo
                            for cc in range(2):
                                nc.tensor.matmul(
                                    ps_q[:, oo, :],
                                    wqkv_sb[:, cc, oc * 128 : (oc + 1) * 128],
                                    xnt_sb[:, cc, 0:TQ],
                                    start=(cc == 0),
                                    stop=(cc == 1),
                                )
                        for oo in range(2):
                            oc = grp * 2 + oo
                            if has_bqkv:
                                nc.vector.tensor_scalar(
                                    out=qt_sb[:, oc, :],
                                    in0=ps_q[:, oo, :],
                                    scalar1=bqk_sb[:, oc : oc + 1],
                                    scalar2=None,
                                    op0=OP.add,
                                )
                            else:
                                nc.vector.tensor_copy(
                                    qt_sb[:, oc, :], ps_q[:, oo, :]
                                )
                # v for this group: evacuate on the scalar engine (idle
                # during LN; the vector engine is the phase-A bottleneck).
                for q in range(4):
                    n = 4 * g + q
                    ps_v = pqv.tile([128, INNER], F32, tag="v", bufs=2)
                    for cc in range(2):
                        nc.tensor.matmul(
                            ps_v,
                            xnt_sb[:, cc, n * 128 : (n + 1) * 128],
                            wqkv_sb[:, cc, 2 * INNER : 3 * INNER],
                            start=(cc == 0),
                            stop=(cc == 1 and not has_bqkv),
                        )
                    if has_bqkv:
                        nc.tensor.matmul(
                            ps_v, ones_tq[:, 0:128], bv_sb, start=False, stop=True
                        )
                    nc.scalar.activation(
                        out=vaug_sb[:, n, :, 0:DH],
                        in_=ps_v[:].rearrange("p (h d) -> p h d", h=H),
                        func=AF.Copy,
                    )

            def emit_kt(oc):
                for half in range(2):
                    ps_k = kb.tile([128, 2, TQ], F32, tag="k", bufs=1)
                    for tt in range(2):
                        tb = half * 2 + tt
                        for cc in range(2):
                            nc.tensor.matmul(
                                ps_k[:, tt, :],
                                wqkv_sb[
                                    :, cc, INNER + oc * 128 : INNER + (oc + 1) * 128
                                ],
                                xnt_sb[:, cc, tb * 512 : (tb + 1) * 512],
                                start=(cc == 0),
                                stop=(cc == 1),
                            )
                    if has_bqkv:
                        nc.vector.tensor_scalar(
                            out=kt_sb[:, oc, half * 1024 : (half + 1) * 1024],
                            in0=ps_k,
                            scalar1=bqk_sb[:, 4 + oc : 5 + oc],
                            scalar2=None,
                            op0=OP.add,
                        )
                    else:
                        nc.vector.tensor_copy(
                            kt_sb[:, oc, half * 1024 : (half + 1) * 1024], ps_k
                        )

            emit_kt(0)
            pqv_cm.__exit__(None, None, None)
            ptr_cm.__exit__(None, None, None)

            # ---- attention: 4 passes x 2 heads ----
            # Unnormalized accumulators are evicted per pass; exp
            # alternates scalar/vector by j parity; kT chunks 1-3 are
            # emitted inside earlier passes to fill PE gaps.
            araw_sb = bigp.tile([68, 4, 2, TQ], F32)
            an_sb = bigp.tile([64, 4, 2, TQ], BF16)
            dnp_sb = bigp.tile([67, 4, 2, TQ], BF16)
            rsp_cm = tc.tile_pool(name="rsp", bufs=2)
            rsp = rsp_cm.__enter__()
            with (
                tc.tile_pool(name="pattn", bufs=2, space="PSUM") as pattn,
                tc.tile_pool(name="expp", bufs=2) as expp,
            ):
                for p in range(4):
                    accum = pattn.tile([68, 2, TQ], F32, tag="accum", bufs=1)
                    for j in range(NT):
                        sT = pattn.tile([128, 2, TQ], F32, tag="sT", bufs=2)
                        for hh in range(2):
                            nc.tensor.matmul(
                                sT[:, hh, :],
                                kt_sb[
                                    hh * 64 : hh * 64 + 64,
                                    p,
                                    j * 128 : (j + 1) * 128,
                                ],
                                qt_sb[hh * 64 : hh * 64 + 64, p, :],
                                start=True,
                                stop=True,
                            )
                        if j % 3 != 2:
                            e = expp.tile([128, 2, TQ], BF16, tag="es", bufs=2)
                            nc.scalar.activation(out=e, in_=sT, func=AF.Exp)
                            eaps = [e[:, 0, :], e[:, 1, :]]
                        else:
                            ei = expp.tile([128, 2, TQ], I16, tag="ev", bufs=2)
                            nc.vector.tensor_scalar(
                                out=ei,
                                in0=sT,
                                scalar1=EXP_A,
                                scalar2=EXP_B,
                                op0=OP.mult,
                                op1=OP.add,
                            )
                            eaps = [
                                ei[:, 0, :].bitcast(BF16),
                                ei[:, 1, :].bitcast(BF16),
                            ]
                        for hh in range(2):
                            h = 2 * p + hh
                            nc.tensor.matmul(
                                accum[:, hh, :],
                                vaug_sb[:, j, h, :],
                                eaps[hh],
                                start=(j == 0),
                                stop=(j == NT - 1),
                            )
                        if j == 5 and p < 3:
                            emit_kt(p + 1)
                    nc.vector.tensor_copy(araw_sb[:, p, :, :], accum)
                    # normalization runs under the next pass: reciprocal
                    # of the ones-row via a small DMA gather, broadcast,
                    # multiply on gpsimd, xyz-subtract, and a tiny DMA
                    # restack of the 3 delta rows to 32*i offsets for the
                    # row-packed MLP.
                    rs = rsp.tile([128, 8], F32, tag="rs")
                    nc.sync.dma_start(out=rs, in_=araw_sb[67:68, p, :, :])
                    rc = rsp.tile([128, 8], F32, tag="rc")
                    nc.vector.reciprocal(out=rc, in_=rs)
                    rrow = rsp.tile([1, 2, TQ], F32, tag="rrow")
                    nc.sync.dma_start(out=rrow, in_=rc)
                    for hh in range(2):
                        rbc = rsp.tile([68, TQ], F32, tag="rbc", bufs=3)
                        nc.gpsimd.partition_broadcast(
                            rbc, rrow[0:1, hh, :], channels=68
                        )
                        nc.vector.tensor_tensor(
                            out=an_sb[:, p, hh, :],
                            in0=araw_sb[0:64, p, hh, :],
                            in1=rbc[0:64, :],
                            op=OP.mult,
                        )
                        dn = dnp_sb[:, p, hh, :]
                        nc.vector.tensor_tensor(
                            out=dn[64:67, :],
                            in0=araw_sb[64:67, p, hh, :],
                            in1=rbc[64:67, :],
                            op=OP.mult,
                        )
                        nc.vector.tensor_tensor(
                            out=dn[64:67, :],
                            in0=dn[64:67, :],
                            in1=xyzt_sb[64:67, :],
                            op=OP.subtract,
                        )
            kb_cm.__exit__(None, None, None)

            # ---- spatial-bias MLP: kc-granular h1 with the h2
            # accumulation interleaved per kc, so the PE fills the gelu
            # shadow. 4 sbias accumulators stay live across the kc loop.
            outfin_sb = bigp.tile([64, H, TQ], BF16)
            with (
                tc.tile_pool(name="pmlp", bufs=1, space="PSUM") as pmlp,
                tc.tile_pool(name="hpool", bufs=2) as hpool,
            ):
                warm2 = pmlp.tile([128, 2, TQ], F32, tag="h1", bufs=2)
                wv = warm2[:].bitcast(BF16)
                for _ in range(20):
                    nc.tensor.transpose(wv[:, 0, 0:128], ident, ident)

                for G in range(2):
                    hsb_G = hpool.tile([128, 4, 4, TQ], BF16, tag="hsb")
                    sb_tiles = [
                        pmlp.tile(
                            [64, TQ], F32, name=f"sb{G}_{i}", tag="sbias", bufs=4
                        )
                        for i in range(4)
                    ]
                    for kc in range(4):
                        for pr in range(2):
                            h1 = pmlp.tile([128, 2, TQ], F32, tag="h1", bufs=2)
                            for ii in range(2):
                                i = 2 * pr + ii
                                h = 4 * G + i
                                ph, hhh = h // 2, h % 2
                                nc.tensor.matmul(
                                    h1[:, ii, :],
                                    spw1_sb[64:67, kc * 128 : (kc + 1) * 128],
                                    dnp_sb[64:67, ph, hhh, :],
                                    start=True,
                                    stop=True,
                                )
                            bias_kc = (
                                spb1_sb[:, kc : kc + 1] if has_spb1 else 0.0
                            )
                            nc.scalar.activation(
                                out=hsb_G[:, 2 * pr : 2 * pr + 2, kc, :],
                                in_=h1,
                                func=AF.Gelu,
                                bias=bias_kc,
                            )
                            for ii in range(2):
                                i = 2 * pr + ii
                                nc.tensor.matmul(
                                    sb_tiles[i],
                                    spw2_sb[:, kc, :],
                                    hsb_G[:, i, kc, :],
                                    start=(kc == 0),
                                    stop=(kc == 3 and not has_spb2),
                                )
                    for i in range(4):
                        h = 4 * G + i
                        if has_spb2:
                            nc.tensor.matmul(
                                sb_tiles[i], spb2_sb, ones_tq, start=False, stop=True
                            )
                        p, hh = h // 2, h % 2
                        nc.vector.tensor_tensor(
                            out=outfin_sb[:, h, :],
                            in0=an_sb[:, p, hh, :],
                            in1=sb_tiles[i],
                            op=OP.add,
                        )

            # ---- output projection + gelu + residual ----
            with tc.tile_pool(name="pproj", bufs=1, space="PSUM") as pproj:
                yT = pproj.tile([128, 2, TQ], F32, tag="y", bufs=1)
                for ec in range(2):
                    for h in range(H):
                        nc.tensor.matmul(
                            yT[:, ec, :],
                            wout_sb[:, h, ec * 128 : (ec + 1) * 128],
                            outfin_sb[:, h, :],
                            start=(h == 0),
                            stop=(h == H - 1),
                        )
                for ec in range(2):
                    ysb = workp.tile([128, TQ], F32, tag="ysb")
                    nc.scalar.activation(
                        out=ysb,
                        in_=yT[:, ec, :],
                        func=AF.Gelu,
                        bias=outb_sb[:, ec : ec + 1],
                    )
                    res = workp.tile([128, TQ], F32, tag="res")
                    nc.vector.tensor_tensor(
                        out=res, in0=ysb, in1=featt_sb[:, ec, :], op=OP.add
                    )
                    nc.sync.dma_start(
                        out=out_d[:].rearrange("(ec p) t -> p ec t", p=128)[:, ec, :],
                        in_=res,
                    )
            rsp_cm.__exit__(None, None, None)

    nc.compile()
    return nc


def prepare_maps(inputs):
    xyzs = np.asarray(inputs["xyzs"], np.float32)
    features = np.asarray(inputs["features"], np.float32)
    ln_g = np.asarray(inputs["ln_g"], np.float32)
    ln_b = np.asarray(inputs["ln_b"], np.float32)
    w_qkv = np.asarray(inputs["w_qkv"], np.float32)
    sp_w1 = np.asarray(inputs["sp_w1"], np.float32)
    sp_b1 = np.asarray(inputs["sp_b1"], np.float32)
    sp_w2 = np.asarray(inputs["sp_w2"], np.float32)
    sp_b2 = np.asarray(inputs["sp_b2"], np.float32)
    out_w = np.asarray(inputs["out_w"], np.float32)
    out_b = np.asarray(inputs["out_b"], np.float32)

    scale = DH ** -0.5
    wqkv_f = w_qkv * ln_g[:, None]
    wqkv_f[:, :INNER] = wqkv_f[:, :INNER] * scale
    bqkv = (ln_b @ w_qkv).astype(np.float32)
    bqkv[:INNER] *= scale

    has_bqkv = bool(np.any(bqkv != 0.0))
    has_spb1 = bool(np.any(sp_b1 != 0.0))
    has_spb2 = bool(np.any(sp_b2 != 0.0))

    cf32 = np.zeros((128, 16), np.float32)
    for oc in range(4):
        cf32[:, oc] = bqkv[oc * 128 : (oc + 1) * 128]
        cf32[:, 4 + oc] = bqkv[INNER + oc * 128 : INNER + (oc + 1) * 128]
    for kc in range(4):
        cf32[:, 8 + kc] = sp_b1[kc * 128 : (kc + 1) * 128]
    cf32[:, 12] = out_b[:128]
    cf32[:, 13] = out_b[128:]

    cbf = np.zeros((1, TQ + INNER + DH), np.float32)
    cbf[0, 0:TQ] = 1.0
    cbf[0, TQ : TQ + INNER] = bqkv[2 * INNER :]
    cbf[0, TQ + INNER :] = sp_b2


    # wout as [64, H, 256]: row (d, h) = out_w[h*64+d, :]
    wout64 = np.ascontiguousarray(out_w.reshape(H, 64, DIM).transpose(1, 0, 2))

    shared = {
        "wqkv": np.ascontiguousarray(wqkv_f).astype(BF),
        "cf32": cf32,
        "cbf": cbf.astype(BF),
        "spw1": np.ascontiguousarray(sp_w1).astype(BF),
        "spw2": np.ascontiguousarray(sp_w2).astype(BF),
        "wout": wout64.astype(BF),
    }

    in_maps = []
    for core in range(N_CORES):
        bi, quarter = core // 4, core % 4
        qs = quarter * TQ
        x_b = features[bi].reshape(M, DIM)
        xyz_b = xyzs[bi].reshape(M, 3)
        x_perm = np.roll(x_b, -qs, axis=0)
        xyz_perm = np.roll(xyz_b, -qs, axis=0)
        xyza = np.concatenate(
            [xyz_perm, np.ones((M, 1), np.float32)], axis=1
        ).astype(np.float32)
        m = dict(shared)
        m["x"] = np.ascontiguousarray(x_perm).astype(BF)
        m["xyzv"] = np.ascontiguousarray(
            xyza.reshape(NT, 128, 4).transpose(1, 0, 2)
        ).astype(BF)
        m["xyzt"] = np.ascontiguousarray(xyz_perm[:TQ].T).astype(BF)
        m["featt"] = np.ascontiguousarray(x_perm[:TQ].T)
        in_maps.append(m)
    return in_maps, (has_bqkv, has_spb1, has_spb2)


def assemble(results, l=16, n=128):
    out = np.zeros((2, M, DIM), np.float32)
    for core in range(N_CORES):
        bi, quarter = core // 4, core % 4
        qs = quarter * TQ
        out[bi, qs : qs + TQ, :] = results[core]["out"].T
    return out.reshape(2, l, n, DIM)


def kernel(**inputs):
    in_maps, flags = prepare_maps(inputs)
    nc = build_program(*flags)
    results = run_bass_kernel_spmd(nc, in_maps, list(range(N_CORES))).results
    return assemble(results)


if __name__ == "__main__":
    pass


# revision 24
# speedup vs baseline: 1.3356x; 1.0277x over previous
"""Trainium2 Bass kernel for nn_Attention_1322849927460.

Dense transformer block: LN -> qkv -> attention (+ spatial-bias MLP on
attention-weighted coordinate deltas) -> out proj -> gelu -> residual.

Sharding: 8 cores = (2 batches) x (4 sequence quarters). Each core holds
all 8 heads for its 512 query rows and the full 2048-token K/V of its
batch, so no collectives are needed. A host-side roll of the token axis
puts each core's query rows first, letting all cores run an identical
SPMD program (attention is invariant to key-order permutation).

Algebraic structure:
  * delta_full[b,h,i,:] = (attn @ xyz)[b,h,i,:] - xyz[b,i,:] since softmax
    rows sum to one -> the (m,m,3) delta tensor is never formed.
  * softmax denominators come free from an augmented V' = [V | xyz | 1]
    contraction; one reciprocal + partition-broadcast normalizes the
    [68, i] accumulator at the end.
  * ln_g and the 1/sqrt(dh) q-scale fold into the qkv weights on host.

Engine split (the point of this version vs the naive schedule):
  * LayerNorm is pipelined per 4-tile group so transposes/qkv chase the
    stats instead of waiting for all 16 tiles.
  * exp of the attention logits alternates between the scalar engine
    (exact ACTIVATE) and the vector engine (Schraudolph fast exp:
    i16 = round(x*128/ln2 + (16256-5.5)), bitcast to bf16), doubling
    softmax throughput. QK logits land in bf16 PSUM so the DVE runs in
    its 2x packed mode.
  * v/qt evacuations run on the scalar engine during the (otherwise
    scalar-idle) LN phase; normalization multiplies run on gpsimd.
  * the spatial-MLP first layer (K=3) packs 4 heads into the PE array
    via tile_position row tiling; gelu is one batched 2048-elem
    ACTIVATE per head.
  * all DMAs issue from the sync queue, keeping the scalar engine free
    for exp/gelu.
"""

import os
import sys

for _p in ("/opt/trn_rl_repo",):
    if _p not in sys.path and os.path.isdir(_p):
        sys.path.insert(0, _p)

import ml_dtypes
import numpy as np

import concourse.bass as bass
import concourse.bacc as bacc
import concourse.tile as tile
from concourse import mybir
from concourse.bass_utils import run_bass_kernel_spmd
from concourse.masks import make_identity

F32 = mybir.dt.float32
BF16 = mybir.dt.bfloat16
I16 = mybir.dt.int16
AF = mybir.ActivationFunctionType
OP = mybir.AluOpType
BF = ml_dtypes.bfloat16

DIM = 256
H = 8
DH = 64
INNER = H * DH  # 512
M = 2048  # tokens per batch
TQ = 512  # query tokens per core
NT = M // 128  # 16 token tiles
N_CORES = 8
LN_EPS = 1e-5

# Schraudolph fast exp in bf16 bit domain:
#   bf16(x) bits = round(x * 2^7/ln2 + (127*2^7 - C)) viewed as int16.
EXP_A = 128.0 / float(np.log(2.0))
EXP_B = 16256.0 - 5.5


def build_program(has_bqkv: bool, has_spb1: bool, has_spb2: bool):
    nc = bacc.Bacc()

    x_d = nc.dram_tensor("x", [M, DIM], BF16, kind="ExternalInput")
    xyzv_d = nc.dram_tensor("xyzv", [128, NT, 4], BF16, kind="ExternalInput")
    xyzt_d = nc.dram_tensor("xyzt", [3, TQ], BF16, kind="ExternalInput")
    featt_d = nc.dram_tensor("featt", [DIM, TQ], F32, kind="ExternalInput")
    wqkv_d = nc.dram_tensor("wqkv", [DIM, 3 * INNER], BF16, kind="ExternalInput")
    spw1_d = nc.dram_tensor("spw1", [3, 2 * DIM], BF16, kind="ExternalInput")
    spw2_d = nc.dram_tensor("spw2", [2 * DIM, DH], BF16, kind="ExternalInput")
    wout_d = nc.dram_tensor("wout", [64, H, DIM], BF16, kind="ExternalInput")
    cf32_d = nc.dram_tensor("cf32", [128, 16], F32, kind="ExternalInput")
    cbf_d = nc.dram_tensor("cbf", [1, TQ + INNER + DH], BF16, kind="ExternalInput")
    out_d = nc.dram_tensor("out", [DIM, TQ], F32, kind="ExternalOutput")

    with tile.TileContext(nc) as tc:
        with (
            tc.tile_pool(name="const", bufs=1) as constp,
            tc.tile_pool(name="big", bufs=1) as bigp,
            tc.tile_pool(name="work", bufs=2) as workp,
        ):
            # ---- DMAs: all on the sync HWDGE queue, critical-path first.
            wqkv_sb = constp.tile([128, 2, 3 * INNER], BF16)
            nc.sync.dma_start(
                out=wqkv_sb, in_=wqkv_d[:].rearrange("(cc p) o -> p cc o", p=128)
            )
            x_sb = bigp.tile([128, NT, DIM], BF16)
            xv = x_d[:].rearrange("(n p) c -> p n c", p=128)
            for g in range(4):
                nc.sync.dma_start(
                    out=x_sb[:, 4 * g : 4 * g + 4, :],
                    in_=xv[:, 4 * g : 4 * g + 4, :],
                )
            xyzv_sb = constp.tile([128, NT, 4], BF16)
            nc.sync.dma_start(out=xyzv_sb, in_=xyzv_d[:])
            xyzt_sb = constp.tile([67, TQ], BF16)
            nc.sync.dma_start(out=xyzt_sb[64:67, :], in_=xyzt_d[:])
            cbf_sb = constp.tile([1, TQ + INNER + DH], BF16)
            nc.sync.dma_start(out=cbf_sb, in_=cbf_d[:])
            cf32_sb = constp.tile([128, 16], F32)
            nc.sync.dma_start(out=cf32_sb, in_=cf32_d[:])
            spw1_sb = constp.tile([67, 2 * DIM], BF16)
            nc.sync.dma_start(out=spw1_sb[64:67, :], in_=spw1_d[:])
            spw2_sb = constp.tile([128, 4, DH], BF16)
            nc.sync.dma_start(
                out=spw2_sb, in_=spw2_d[:].rearrange("(kc p) d -> p kc d", p=128)
            )
            wout_sb = constp.tile([64, H, DIM], BF16)
            nc.sync.dma_start(out=wout_sb, in_=wout_d[:])
            featt_sb = constp.tile([128, 2, TQ], F32)
            nc.sync.dma_start(
                out=featt_sb, in_=featt_d[:].rearrange("(ec p) t -> p ec t", p=128)
            )

            ones_tq = cbf_sb[0:1, 0:TQ]
            bv_sb = cbf_sb[0:1, TQ : TQ + INNER]
            spb2_sb = cbf_sb[0:1, TQ + INNER : TQ + INNER + DH]
            bqk_sb = cf32_sb[:, 0:8]
            spb1_sb = cf32_sb[:, 8:12]
            outb_sb = cf32_sb[:, 12:14]

            ident = constp.tile([128, 128], BF16)
            make_identity(nc, ident)
            eps_t = constp.tile([128, 1], F32)
            nc.vector.memset(eps_t, LN_EPS)

            # xyz|ones columns of Vaug: one compact DMA + strided gpsimd
            # copies per head (avoids a 16k-packet strided DMA).
            vaug_sb = bigp.tile([128, NT, H, DH + 4], BF16)
            for h in range(H):
                nc.gpsimd.tensor_copy(vaug_sb[:, :, h, DH : DH + 4], xyzv_sb)

            # PE priming: absorb one DMA-queue semaphore per DMA-loaded
            # tile the PE consumes, so real matmuls stay under the
            # per-instruction sync-wait limit. Plus warm spam to open the
            # HAM clock gate during the DMA lead-in.
            # Pool nesting (LIFO): kb outlives ptr/pqv (attention reuses it).
            kb_cm = tc.tile_pool(name="kb", bufs=1, space="PSUM")
            kb = kb_cm.__enter__()
            ptr_cm = tc.tile_pool(name="ptr", bufs=2, space="PSUM")
            ptr = ptr_cm.__enter__()
            warm_ps = ptr.tile([128, 128], BF16, tag="warm", bufs=1)

            def warm(n):
                for _ in range(n):
                    nc.tensor.transpose(warm_ps, ident, ident)

            warm(24)
            prime_ps = ptr.tile([4, 4], F32, tag="prime", bufs=1)

            def prime(lhsT, rhs):
                nc.tensor.matmul(
                    prime_ps[0 : lhsT.shape[-1], 0 : rhs.shape[-1]],
                    lhsT,
                    rhs,
                    start=True,
                    stop=True,
                )

            prime(wqkv_sb[:, 0, 0:4], wqkv_sb[:, 0, 0:4])
            prime(spw1_sb[64:67, 0:4], spw1_sb[64:67, 0:4])
            prime(spw2_sb[:, 0, 0:4], spw2_sb[:, 0, 0:4])
            prime(wout_sb[:, 0, 0:4], wout_sb[:, 0, 0:4])
            if has_bqkv:
                prime(ones_tq[:, 0:4], bv_sb[:, 0:4])
            if has_spb2:
                prime(spb2_sb[:, 0:4], ones_tq[:, 0:4])
            warm(12)

            # ---- Phase A: LN -> transpose -> q/kt0/v, pipelined per
            # 4-tile group.
            xn_sb = bigp.tile([128, NT, DIM], BF16)
            xnt_sb = bigp.tile([128, 2, M], BF16)
            qt_sb = bigp.tile([128, 4, TQ], BF16)
            kt_sb = bigp.tile([128, 4, M], BF16)
            mv_all = constp.tile([128, NT, 2], F32)
            rstd = constp.tile([128, NT], F32)

            pqv_cm = tc.tile_pool(name="pqv", bufs=2, space="PSUM")
            pqv = pqv_cm.__enter__()

            for g in range(4):
                for q in range(4):
                    n = 4 * g + q
                    stats = workp.tile([128, 6], F32, tag="bnstats")
                    nc.vector.bn_stats(out=stats, in_=x_sb[:, n, :])
                    nc.vector.bn_aggr(out=mv_all[:, n, :], in_=stats)
                nc.scalar.activation(
                    out=rstd[:, 4 * g : 4 * g + 4],
                    in_=mv_all[:, 4 * g : 4 * g + 4, 1],
                    func=AF.Sqrt,
                    bias=eps_t,
                    scale=1.0,
                )
                nc.vector.reciprocal(
                    out=rstd[:, 4 * g : 4 * g + 4],
                    in_=rstd[:, 4 * g : 4 * g + 4],
                )
                for q in range(4):
                    n = 4 * g + q
                    nc.vector.tensor_scalar(
                        out=xn_sb[:, n, :],
                        in0=x_sb[:, n, :],
                        scalar1=mv_all[:, n, 0:1],
                        scalar2=rstd[:, n : n + 1],
                        op0=OP.subtract,
                        op1=OP.mult,
                    )
                # transpose this group into xnT
                for cc in range(2):
                    ps = ptr.tile([128, 512], BF16, tag="tr")
                    for q in range(4):
                        n = 4 * g + q
                        nc.tensor.transpose(
                            ps[:, q * 128 : (q + 1) * 128],
                            xn_sb[:, n, cc * 128 : (cc + 1) * 128],
                            ident,
                        )
                    nc.vector.tensor_copy(
                        xnt_sb[:, cc, g * 512 : (g + 1) * 512], ps
                    )
                if g == 0:
                    # q projection for this core's 512 queries; borrows
                    # the kb pool's banks (emit_kt only runs after g3).
                    for grp in range(2):
                        ps_q = kb.tile([128, 2, TQ], F32, tag="k", bufs=1)
                        for oo in range(2):
                            oc = grp * 2 + oo
                            for cc in range(2):
                                nc.tensor.matmul(
                                    ps_q[:, oo, :],
                                    wqkv_sb[:, cc, oc * 128 : (oc + 1) * 128],
                                    xnt_sb[:, cc, 0:TQ],
                                    start=(cc == 0),
                                    stop=(cc == 1),
                                )
                        for oo in range(2):
                            oc = grp * 2 + oo
                            if has_bqkv:
                                nc.vector.tensor_scalar(
                                    out=qt_sb[:, oc, :],
                                    in0=ps_q[:, oo, :],
                                    scalar1=bqk_sb[:, oc : oc + 1],
                                    scalar2=None,
                                    op0=OP.add,
                                )
                            else:
                                nc.vector.tensor_copy(
                                    qt_sb[:, oc, :], ps_q[:, oo, :]
                                )
                # v for this group: evacuate on the scalar engine (idle
                # during LN; the vector engine is the phase-A bottleneck).
                for q in range(4):
                    n = 4 * g + q
                    ps_v = pqv.tile([128, INNER], F32, tag="v", bufs=2)
                    for cc in range(2):
                        nc.tensor.matmul(
                            ps_v,
                            xnt_sb[:, cc, n * 128 : (n + 1) * 128],
                            wqkv_sb[:, cc, 2 * INNER : 3 * INNER],
                            start=(cc == 0),
                            stop=(cc == 1 and not has_bqkv),
                        )
                    if has_bqkv:
                        nc.tensor.matmul(
                            ps_v, ones_tq[:, 0:128], bv_sb, start=False, stop=True
                        )
                    nc.scalar.activation(
                        out=vaug_sb[:, n, :, 0:DH],
                        in_=ps_v[:].rearrange("p (h d) -> p h d", h=H),
                        func=AF.Copy,
                    )

            def emit_kt(oc):
                for half in range(2):
                    ps_k = kb.tile([128, 2, TQ], F32, tag="k", bufs=1)
                    for tt in range(2):
                        tb = half * 2 + tt
                        for cc in range(2):
                            nc.tensor.matmul(
                                ps_k[:, tt, :],
                                wqkv_sb[
                                    :, cc, INNER + oc * 128 : INNER + (oc + 1) * 128
                                ],
                                xnt_sb[:, cc, tb * 512 : (tb + 1) * 512],
                                start=(cc == 0),
                                stop=(cc == 1),
                            )
                    if has_bqkv:
                        nc.vector.tensor_scalar(
                            out=kt_sb[:, oc, half * 1024 : (half + 1) * 1024],
                            in0=ps_k,
                            scalar1=bqk_sb[:, 4 + oc : 5 + oc],
                            scalar2=None,
                            op0=OP.add,
                        )
                    else:
                        nc.vector.tensor_copy(
                            kt_sb[:, oc, half * 1024 : (half + 1) * 1024], ps_k
                        )

            emit_kt(0)
            pqv_cm.__exit__(None, None, None)
            ptr_cm.__exit__(None, None, None)

            # ---- attention: 4 passes x 2 heads ----
            # Unnormalized accumulators are evicted per pass; exp
            # alternates scalar/vector by j parity; kT chunks 1-3 are
            # emitted inside earlier passes to fill PE gaps.
            araw_sb = bigp.tile([68, 4, 2, TQ], F32)
            an_sb = bigp.tile([64, 4, 2, TQ], BF16)
            dnp_sb = bigp.tile([67, 4, 2, TQ], BF16)
            rsp_cm = tc.tile_pool(name="rsp", bufs=2)
            rsp = rsp_cm.__enter__()
            with (
                tc.tile_pool(name="pattn", bufs=2, space="PSUM") as pattn,
                tc.tile_pool(name="expp", bufs=2) as expp,
            ):
                def qk_pair(p, j):
                    sT = pattn.tile([128, 2, TQ], F32, tag="sT", bufs=2)
                    for hh in range(2):
                        nc.tensor.matmul(
                            sT[:, hh, :],
                            kt_sb[
                                hh * 64 : hh * 64 + 64,
                                p,
                                j * 128 : (j + 1) * 128,
                            ],
                            qt_sb[hh * 64 : hh * 64 + 64, p, :],
                            start=True,
                            stop=True,
                        )
                    return sT

                for p in range(4):
                    accum = pattn.tile([68, 2, TQ], F32, tag="accum", bufs=1)
                    # software pipeline: QK for tile j+1 issues ahead of
                    # AV_j on the in-order PE queue, so the next exp
                    # (alternating engines) overlaps the current one
                    # instead of serializing behind the AV dependency.
                    sT = qk_pair(p, 0)
                    for j in range(NT):
                        sT_next = qk_pair(p, j + 1) if j + 1 < NT else None
                        if j % 3 != 2:
                            e = expp.tile([128, 2, TQ], BF16, tag="es", bufs=2)
                            nc.scalar.activation(out=e, in_=sT, func=AF.Exp)
                            eaps = [e[:, 0, :], e[:, 1, :]]
                        else:
                            ei = expp.tile([128, 2, TQ], I16, tag="ev", bufs=2)
                            nc.vector.tensor_scalar(
                                out=ei,
                                in0=sT,
                                scalar1=EXP_A,
                                scalar2=EXP_B,
                                op0=OP.mult,
                                op1=OP.add,
                            )
                            eaps = [
                                ei[:, 0, :].bitcast(BF16),
                                ei[:, 1, :].bitcast(BF16),
                            ]
                        for hh in range(2):
                            h = 2 * p + hh
                            nc.tensor.matmul(
                                accum[:, hh, :],
                                vaug_sb[:, j, h, :],
                                eaps[hh],
                                start=(j == 0),
                                stop=(j == NT - 1),
                            )
                        if j == 5 and p < 3:
                            emit_kt(p + 1)
                        sT = sT_next
                    if p == 3:
                        # keep the PE busy through the norm chain + pool
                        # handoff so HAM stays at full clock into the MLP.
                        wps = pattn.tile([128, 2, TQ], F32, tag="sT", bufs=2)
                        wv = wps[:, 0, 0:64].bitcast(BF16)
                        for _ in range(30):
                            nc.tensor.transpose(wv, ident, ident)
                    nc.vector.tensor_copy(araw_sb[:, p, :, :], accum)
                    # normalization runs under the next pass: reciprocal
                    # of the ones-row via a small DMA gather, broadcast,
                    # multiply on gpsimd, xyz-subtract, and a tiny DMA
                    # restack of the 3 delta rows to 32*i offsets for the
                    # row-packed MLP.
                    rs = rsp.tile([128, 8], F32, tag="rs")
                    nc.sync.dma_start(out=rs, in_=araw_sb[67:68, p, :, :])
                    rc = rsp.tile([128, 8], F32, tag="rc")
                    nc.vector.reciprocal(out=rc, in_=rs)
                    rrow = rsp.tile([1, 2, TQ], F32, tag="rrow")
                    nc.sync.dma_start(out=rrow, in_=rc)
                    for hh in range(2):
                        rbc = rsp.tile([68, TQ], F32, tag="rbc", bufs=3)
                        nc.gpsimd.partition_broadcast(
                            rbc, rrow[0:1, hh, :], channels=68
                        )
                        nc.vector.tensor_tensor(
                            out=an_sb[:, p, hh, :],
                            in0=araw_sb[0:64, p, hh, :],
                            in1=rbc[0:64, :],
                            op=OP.mult,
                        )
                        dn = dnp_sb[:, p, hh, :]
                        nc.vector.tensor_tensor(
                            out=dn[64:67, :],
                            in0=araw_sb[64:67, p, hh, :],
                            in1=rbc[64:67, :],
                            op=OP.mult,
                        )
                        nc.vector.tensor_tensor(
                            out=dn[64:67, :],
                            in0=dn[64:67, :],
                            in1=xyzt_sb[64:67, :],
                            op=OP.subtract,
                        )
            kb_cm.__exit__(None, None, None)

            # ---- spatial-bias MLP: kc-granular h1 with the h2
            # accumulation interleaved per kc, so the PE fills the gelu
            # shadow. 4 sbias accumulators stay live across the kc loop.
            outfin_sb = bigp.tile([64, H, TQ], BF16)
            with (
                tc.tile_pool(name="pmlp", bufs=1, space="PSUM") as pmlp,
                tc.tile_pool(name="hpool", bufs=2) as hpool,
            ):
                warm2 = pmlp.tile([128, 2, TQ], F32, tag="h1", bufs=2)
                wv = warm2[:].bitcast(BF16)
                for _ in range(20):
                    nc.tensor.transpose(wv[:, 0, 0:128], ident, ident)

                for G in range(2):
                    hsb_G = hpool.tile([128, 4, 4, TQ], BF16, tag="hsb")
                    sb_tiles = [
                        pmlp.tile(
                            [64, TQ], F32, name=f"sb{G}_{i}", tag="sbias", bufs=4
                        )
                        for i in range(4)
                    ]
                    for kc in range(4):
                        for pr in range(2):
                            h1 = pmlp.tile([128, 2, TQ], F32, tag="h1", bufs=2)
                            for ii in range(2):
                                i = 2 * pr + ii
                                h = 4 * G + i
                                ph, hhh = h // 2, h % 2
                                nc.tensor.matmul(
                                    h1[:, ii, :],
                                    spw1_sb[64:67, kc * 128 : (kc + 1) * 128],
                                    dnp_sb[64:67, ph, hhh, :],
                                    start=True,
                                    stop=True,
                                )
                            bias_kc = (
                                spb1_sb[:, kc : kc + 1] if has_spb1 else 0.0
                            )
                            nc.scalar.activation(
                                out=hsb_G[:, 2 * pr : 2 * pr + 2, kc, :],
                                in_=h1,
                                func=AF.Gelu,
                                bias=bias_kc,
                            )
                            for ii in range(2):
                                i = 2 * pr + ii
                                nc.tensor.matmul(
                                    sb_tiles[i],
                                    spw2_sb[:, kc, :],
                                    hsb_G[:, i, kc, :],
                                    start=(kc == 0),
                                    stop=(kc == 3 and not has_spb2),
                                )
                    for i in range(4):
                        h = 4 * G + i
                        if has_spb2:
                            nc.tensor.matmul(
                                sb_tiles[i], spb2_sb, ones_tq, start=False, stop=True
                            )
                        p, hh = h // 2, h % 2
                        nc.vector.tensor_tensor(
                            out=outfin_sb[:, h, :],
                            in0=an_sb[:, p, hh, :],
                            in1=sb_tiles[i],
                            op=OP.add,
                        )

            # ---- output projection + gelu + residual ----
            with tc.tile_pool(name="pproj", bufs=1, space="PSUM") as pproj:
                yT = pproj.tile([128, 2, TQ], F32, tag="y", bufs=1)
                for ec in range(2):
                    for h in range(H):
                        nc.tensor.matmul(
                            yT[:, ec, :],
                            wout_sb[:, h, ec * 128 : (ec + 1) * 128],
                            outfin_sb[:, h, :],
                            start=(h == 0),
                            stop=(h == H - 1),
                        )
                for ec in range(2):
                    ysb = workp.tile([128, TQ], F32, tag="ysb")
                    nc.scalar.activation(
                        out=ysb,
                        in_=yT[:, ec, :],
                        func=AF.Gelu,
                        bias=outb_sb[:, ec : ec + 1],
                    )
                    res = workp.tile([128, TQ], F32, tag="res")
                    nc.vector.tensor_tensor(
                        out=res, in0=ysb, in1=featt_sb[:, ec, :], op=OP.add
                    )
                    nc.sync.dma_start(
                        out=out_d[:].rearrange("(ec p) t -> p ec t", p=128)[:, ec, :],
                        in_=res,
                    )
            rsp_cm.__exit__(None, None, None)

    nc.compile()
    return nc


def prepare_maps(inputs):
    xyzs = np.asarray(inputs["xyzs"], np.float32)
    features = np.asarray(inputs["features"], np.float32)
    ln_g = np.asarray(inputs["ln_g"], np.float32)
    ln_b = np.asarray(inputs["ln_b"], np.float32)
    w_qkv = np.asarray(inputs["w_qkv"], np.float32)
    sp_w1 = np.asarray(inputs["sp_w1"], np.float32)
    sp_b1 = np.asarray(inputs["sp_b1"], np.float32)
    sp_w2 = np.asarray(inputs["sp_w2"], np.float32)
    sp_b2 = np.asarray(inputs["sp_b2"], np.float32)
    out_w = np.asarray(inputs["out_w"], np.float32)
    out_b = np.asarray(inputs["out_b"], np.float32)

    scale = DH ** -0.5
    wqkv_f = w_qkv * ln_g[:, None]
    wqkv_f[:, :INNER] = wqkv_f[:, :INNER] * scale
    bqkv = (ln_b @ w_qkv).astype(np.float32)
    bqkv[:INNER] *= scale

    has_bqkv = bool(np.any(bqkv != 0.0))
    has_spb1 = bool(np.any(sp_b1 != 0.0))
    has_spb2 = bool(np.any(sp_b2 != 0.0))

    cf32 = np.zeros((128, 16), np.float32)
    for oc in range(4):
        cf32[:, oc] = bqkv[oc * 128 : (oc + 1) * 128]
        cf32[:, 4 + oc] = bqkv[INNER + oc * 128 : INNER + (oc + 1) * 128]
    for kc in range(4):
        cf32[:, 8 + kc] = sp_b1[kc * 128 : (kc + 1) * 128]
    cf32[:, 12] = out_b[:128]
    cf32[:, 13] = out_b[128:]

    cbf = np.zeros((1, TQ + INNER + DH), np.float32)
    cbf[0, 0:TQ] = 1.0
    cbf[0, TQ : TQ + INNER] = bqkv[2 * INNER :]
    cbf[0, TQ + INNER :] = sp_b2


    # wout as [64, H, 256]: row (d, h) = out_w[h*64+d, :]
    wout64 = np.ascontiguousarray(out_w.reshape(H, 64, DIM).transpose(1, 0, 2))

    shared = {
        "wqkv": np.ascontiguousarray(wqkv_f).astype(BF),
        "cf32": cf32,
        "cbf": cbf.astype(BF),
        "spw1": np.ascontiguousarray(sp_w1).astype(BF),
        "spw2": np.ascontiguousarray(sp_w2).astype(BF),
        "wout": wout64.astype(BF),
    }

    in_maps = []
    for core in range(N_CORES):
        bi, quarter = core // 4, core % 4
        qs = quarter * TQ
        x_b = features[bi].reshape(M, DIM)
        xyz_b = xyzs[bi].reshape(M, 3)
        x_perm = np.roll(x_b, -qs, axis=0)
        xyz_perm = np.roll(xyz_b, -qs, axis=0)
        xyza = np.concatenate(
            [xyz_perm, np.ones((M, 1), np.float32)], axis=1
        ).astype(np.float32)
        m = dict(shared)
        m["x"] = np.ascontiguousarray(x_perm).astype(BF)
        m["xyzv"] = np.ascontiguousarray(
            xyza.reshape(NT, 128, 4).transpose(1, 0, 2)
        ).astype(BF)
        m["xyzt"] = np.ascontiguousarray(xyz_perm[:TQ].T).astype(BF)
        m["featt"] = np.ascontiguousarray(x_perm[:TQ].T)
        in_maps.append(m)
    return in_maps, (has_bqkv, has_spb1, has_spb2)


def assemble(results, l=16, n=128):
    out = np.zeros((2, M, DIM), np.float32)
    for core in range(N_CORES):
        bi, quarter = core // 4, core % 4
        qs = quarter * TQ
        out[bi, qs : qs + TQ, :] = results[core]["out"].T
    return out.reshape(2, l, n, DIM)


def kernel(**inputs):
    in_maps, flags = prepare_maps(inputs)
    nc = build_program(*flags)
    results = run_bass_kernel_spmd(nc, in_maps, list(range(N_CORES))).results
    return assemble(results)


if __name__ == "__main__":
    pass
